# revision 19
# baseline (speedup 1.0000x reference)
"""Multi-head attention block for Trainium2, 8-core data-parallel SPMD.

Computes, per batch element b (one NeuronCore each):
    qkv = x @ w_qkv ; q,k,v split into 16 heads of dim 64
    attn = softmax(q @ k^T / sqrt(64)) ; out = (attn @ v) @ w_out + b_out

Strategy (per core):
  - transpose x -> xT (c-major) via PE transposes
  - v computed in natural layout, written strided into v_aug tiles with a
    ones-column per head so the attention output matmul also produces the
    softmax row-sums for free
  - attention per head in transposed layout: s^T = kT^T @ qT on the PE,
    exp on ACT (1/8 scale folded in), o^T_aug accumulated over k chunks;
    softmax normalization deferred to o^T (DVE reciprocal + K=1 ones-matmul
    partition-broadcast)
  - the qT/kT projection of head pair t+1 is explicitly interleaved into
    the attention instruction stream of pair t (engines execute their
    streams in order, so overlap has to be emitted, not just scheduled)
  - out = o^T^T @ w_out + ones x b_out (bias added by the PE)
All matmul-feeding tiles are declared float32r (full PE rate; the producing
DVE/ACT/DMA instructions emit the FP32r rounding the BIR verifier requires).
"""

import sys

if "/opt/trn_rl_repo" not in sys.path:
    sys.path.insert(0, "/opt/trn_rl_repo")

import numpy as np

B = 8
N = 1024  # sequence length
C = 1024  # model dim
H = 16  # heads
D = 64  # head dim
P = 128  # partitions
NT = N // P  # seq chunks
CT = C // P  # channel chunks
HP = H // 2  # head pairs
SCALE = D ** -0.5
HF = C // 512  # free-dim halves per 1024 row

_CACHE = {}


def _build_program():
    from concourse import bacc, mybir
    import concourse.tile as tile
    from concourse.masks import make_identity

    f32 = mybir.dt.float32
    f32r = mybir.dt.float32r
    Exp = mybir.ActivationFunctionType.Exp

    nc = bacc.Bacc("TRN2", target_bir_lowering=False, debug=False)
    x_d = nc.declare_dram_parameter("x", [N, C], f32r, isOutput=False)
    wqkv_d = nc.declare_dram_parameter("w_qkv", [C, 3 * C], f32r, isOutput=False)
    wout_d = nc.declare_dram_parameter("w_out", [C, C], f32r, isOutput=False)
    bout_d = nc.declare_dram_parameter("b_out", [1, C], f32r, isOutput=False)
    out_d = nc.declare_dram_parameter("out", [N, C], f32, isOutput=True)

    with tile.TileContext(nc) as tc:
        with (
            tc.tile_pool(name="consts", bufs=1) as consts,
            tc.tile_pool(name="xTo", bufs=CT) as xT_pool,
            tc.tile_pool(name="vaug", bufs=NT) as vaug_pool,
            tc.tile_pool(name="psum", bufs=1, space="PSUM") as psum,
            tc.tile_pool(name="oTp", bufs=CT) as oT_pool,
            tc.tile_pool(name="io", bufs=3) as io_pool,
            tc.tile_pool(name="w", bufs=CT) as w_pool,
            tc.tile_pool(name="wqk", bufs=4) as wqk_pool,
            tc.tile_pool(name="pT", bufs=8) as pT_pool,
            tc.tile_pool(name="recip", bufs=1) as recip_pool,
            tc.tile_pool(name="bcs", bufs=1) as bcs_pool,
            tc.tile_pool(name="qkT", bufs=4) as qkT_pool,
        ):
            identity_f32 = consts.tile(
                [P, P], f32, name="identity_f32", tag="identity_f32"
            )
            make_identity(nc, identity_f32)
            # f32r transpose runs 1.5 PE cycles/row vs 2.0 for f32
            identity = consts.tile([P, P], f32r, name="identity", tag="identity")
            nc.vector.tensor_copy(identity[:, :], identity_f32[:, :])
            # memset can't emit f32r (ISA check) — stage in f32, round via copy
            ones_f32 = consts.tile([P, P], f32, name="ones_f32", tag="ones_f32")
            nc.vector.memset(ones_f32, 1.0)
            ones = consts.tile([1, P], f32r, name="ones", tag="ones")
            nc.vector.tensor_copy(ones[0:1, :], ones_f32[0:1, :])
            b_row = consts.tile([1, C], f32r, name="b_row", tag="b_row")
            nc.sync.dma_start(out=b_row[0:1, :], in_=bout_d[0:1, :])

            xT = [
                xT_pool.tile([P, N], f32r, name=f"xT{i}", tag="xTo") for i in range(CT)
            ]
            vaug = [
                vaug_pool.tile([P, H * (D + 1)], f32r, name=f"vaug{i}", tag="vaug")
                for i in range(NT)
            ]

            def mm_tile(name, tag, bufs):
                return psum.tile([P, C], f32, name=name, tag=tag, bufs=bufs)

            def half_tile(name, tag, bufs):
                return psum.tile([P, 512], f32, name=name, tag=tag, bufs=bufs)

            # ---------------- phase 0: transpose x into xT ----------------
            for si in range(NT):
                xin = io_pool.tile([P, C], f32r, name=f"xin{si}", tag="io")
                nc.sync.dma_start(out=xin[:, :], in_=x_d[si * P : (si + 1) * P, :])
                tr_ps = psum.tile([P, C], f32r, name=f"tr{si}", tag="mm", bufs=2, padded_shape=None)
                for ci in range(CT):
                    nc.tensor.transpose(
                        tr_ps[:, ci * P : (ci + 1) * P],
                        xin[:, ci * P : (ci + 1) * P],
                        identity,
                    )
                for ci in range(CT):
                    nc.vector.tensor_copy(
                        xT[ci][:, si * P : (si + 1) * P],
                        tr_ps[:, ci * P : (ci + 1) * P],
                    )

            # ---------- phase 1B: v (natural layout) -> v_aug ----------
            # 4 seq-chunks per pass (2x [P,C] from mm/acc tags + 2 halves in
            # the sT slots) -> w_v rows streamed only twice.
            for sc0 in range(0, NT, 4):
                scs = list(range(sc0, sc0 + 4))
                full = {scs[0]: mm_tile(f"vps{scs[0]}", "mm", 2),
                        scs[1]: mm_tile(f"vps{scs[1]}", "mm", 2),
                        scs[2]: mm_tile(f"vps{scs[2]}", "acc", 1)}
                sc3 = scs[3]
                halves = [
                    half_tile(f"vps{sc3}_0", "sT", 2),
                    half_tile(f"vps{sc3}_1", "sT", 2),
                ]
                for ci in range(CT):
                    wv = w_pool.tile([P, C], f32r, name=f"wv{sc0}_{ci}", tag="w")
                    nc.sync.dma_start(
                        out=wv[:, :],
                        in_=wqkv_d[ci * P : (ci + 1) * P, 2 * C : 3 * C],
                    )
                    st = dict(start=(ci == 0), stop=(ci == CT - 1))
                    for hf in range(HF):
                        sl = slice(hf * 512, hf * 512 + 512)
                        for sc in scs[:3]:
                            nc.tensor.matmul(
                                full[sc][:, sl],
                                xT[ci][:, sc * P : (sc + 1) * P],
                                wv[:, sl],
                                **st,
                            )
                        nc.tensor.matmul(
                            halves[hf][:, :],
                            xT[ci][:, sc3 * P : (sc3 + 1) * P],
                            wv[:, sl],
                            **st,
                        )
                for sc in scs[:3]:
                    va3 = vaug[sc].rearrange("p (h u) -> p h u", u=D + 1)
                    nc.vector.tensor_copy(
                        va3[:, :, D : D + 1],
                        ones_f32[:, 0:H].rearrange("p (h u) -> p h u", u=1),
                    )
                    nc.vector.tensor_copy(
                        va3[:, :, 0:D],
                        full[sc].rearrange("p (h u) -> p h u", u=D),
                    )
                va3 = vaug[sc3].rearrange("p (h u) -> p h u", u=D + 1)
                nc.vector.tensor_copy(
                    va3[:, :, D : D + 1],
                    ones_f32[:, 0:H].rearrange("p (h u) -> p h u", u=1),
                )
                for hf in range(HF):
                    nc.vector.tensor_copy(
                        va3[:, 8 * hf : 8 * hf + 8, 0:D],
                        halves[hf].rearrange("p (h u) -> p h u", u=D),
                    )

            # ---- interleaved: attention pair t || qT/kT projection pair t+1 ----
            def qkv_pair_steps(t, qTt, kTt, q_ps, k_ps):
                """Generator: one ci-step (2 weight DMAs + 4 matmuls) per next();
                finishes with the PSUM->SBUF copies."""
                for ci in range(CT):
                    wq = wqk_pool.tile([P, P], f32r, name=f"wq{t}_{ci}", tag="wqk")
                    nc.sync.dma_start(
                        out=wq[:, :],
                        in_=wqkv_d[ci * P : (ci + 1) * P, t * P : (t + 1) * P],
                    )
                    wk = wqk_pool.tile([P, P], f32r, name=f"wk{t}_{ci}", tag="wqk")
                    nc.sync.dma_start(
                        out=wk[:, :],
                        in_=wqkv_d[ci * P : (ci + 1) * P, C + t * P : C + (t + 1) * P],
                    )
                    st = dict(start=(ci == 0), stop=(ci == CT - 1))
                    for hf in range(HF):
                        sl = slice(hf * 512, hf * 512 + 512)
                        nc.tensor.matmul(q_ps[:, sl], wq[:, :], xT[ci][:, sl], **st)
                        nc.tensor.matmul(k_ps[:, sl], wk[:, :], xT[ci][:, sl], **st)
                    yield
                nc.vector.tensor_copy(qTt[:, :], q_ps[:, :])
                nc.vector.tensor_copy(kTt[:, :], k_ps[:, :])
                yield

            def new_pair_qkv(t):
                qTt = qkT_pool.tile([P, N], f32r, name=f"qT{t}", tag="qkT")
                kTt = qkT_pool.tile([P, N], f32r, name=f"kT{t}", tag="qkT")
                q_ps = mm_tile(f"qps{t}", "mm", 2)
                k_ps = mm_tile(f"kps{t}", "mm", 2)
                return qTt, kTt, qkv_pair_steps(t, qTt, kTt, q_ps, k_ps)

            oT = [
                oT_pool.tile([P, N], f32r, name=f"oT{i}", tag="oTp")
                for i in range(CT)
            ]

            # prologue: pair 0 projection emitted straight
            qT_cur, kT_cur, gen0 = new_pair_qkv(0)
            for _ in gen0:
                pass

            # w_out is prefetched one row-chunk per head pair (inside the
            # pair loop) so the DMAs spread across the attention region
            wos = []

            def prefetch_wo(ci):
                wo = w_pool.tile([P, C], f32r, name=f"wo{ci}", tag="w")
                nc.sync.dma_start(out=wo[:, :], in_=wout_d[ci * P : (ci + 1) * P, :])
                wos.append(wo)

            for t in range(HP):
                prefetch_wo(t)
                if t + 1 < HP:
                    qT_nxt, kT_nxt, gen = new_pair_qkv(t + 1)
                else:
                    qT_nxt = kT_nxt = gen = None
                chunk_idx = 0
                for j in range(2):
                    h = 2 * t + j
                    row0 = D * j
                    acc = mm_tile(f"acc{h}", "acc", 1)
                    for kc in range(NT):
                        for hf in range(HF):
                            sl = slice(hf * 512, hf * 512 + 512)
                            s_ps = half_tile(f"s{h}_{kc}_{hf}", "sT", 2)
                            nc.tensor.matmul(
                                s_ps[:, :],
                                kT_cur[row0 : row0 + D, kc * P : (kc + 1) * P],
                                qT_cur[row0 : row0 + D, sl],
                                start=True,
                                stop=True,
                            )
                            pt = pT_pool.tile(
                                [P, 512], f32r, name=f"pt{h}_{kc}_{hf}", tag="pT"
                            )
                            nc.scalar.activation(
                                out=pt[:, :], in_=s_ps[:, :], func=Exp, scale=SCALE
                            )
                            nc.tensor.matmul(
                                acc[0 : D + 1, sl],
                                vaug[kc][:, h * (D + 1) : (h + 1) * (D + 1)],
                                pt[:, :],
                                start=(kc == 0),
                                stop=(kc == NT - 1),
                            )
                            # sprinkle next pair's projection into the stream
                            if gen is not None and chunk_idx % 3 == 2:
                                next(gen, None)
                            chunk_idx += 1
                    # normalize: o^T[d, q] *= 1 / rowsum[q]
                    rc = recip_pool.tile([1, N], f32r, name=f"rc{h}", tag="recip")
                    with nc.allow_low_precision(
                        reason="softmax norm reciprocal rounded to f32r "
                        "for the PE broadcast matmul"
                    ):
                        nc.vector.reciprocal(rc[0:1, :], acc[D : D + 1, :])
                    bcs = bcs_pool.tile([D, N], f32, name=f"bcs{h}", tag="bcs")
                    for hf in range(HF):
                        sl = slice(hf * 512, hf * 512 + 512)
                        bc = half_tile(f"bc{h}_{hf}", "sT", 2)
                        nc.tensor.matmul(
                            bc[0:D, :],
                            ones[0:1, 0:D],
                            rc[0:1, sl],
                            start=True,
                            stop=True,
                        )
                        # DVE reads at most one PSUM operand: stage in SBUF
                        nc.vector.tensor_copy(bcs[0:D, sl], bc[0:D, :])
                    nc.vector.tensor_mul(
                        oT[t][row0 : row0 + D, :],
                        acc[0:D, :],
                        bcs[0:D, :],
                    )
                if gen is not None:
                    for _ in gen:
                        pass
                qT_cur, kT_cur = qT_nxt, kT_nxt

            # ---------------- phase 3: out = o @ w_out + b ----------------
            for sc in range(NT):
                o_ps = mm_tile(f"ops{sc}", "mm", 2)
                for ci in range(CT):
                    for hf in range(HF):
                        sl = slice(hf * 512, hf * 512 + 512)
                        nc.tensor.matmul(
                            o_ps[:, sl],
                            oT[ci][:, sc * P : (sc + 1) * P],
                            wos[ci][:, sl],
                            start=(ci == 0),
                            stop=False,
                        )
                for hf in range(HF):
                    sl = slice(hf * 512, hf * 512 + 512)
                    nc.tensor.matmul(
                        o_ps[:, sl],
                        ones[0:1, 0:P],
                        b_row[0:1, sl],
                        start=False,
                        stop=True,
                    )
                ot = io_pool.tile([P, C], f32, name=f"ot{sc}", tag="io")
                nc.vector.tensor_copy(ot[:, :], o_ps[:, :])
                nc.sync.dma_start(out=out_d[sc * P : (sc + 1) * P, :], in_=ot[:, :])

    nc.compile()
    return nc


def _get_program():
    if "nc" not in _CACHE:
        _CACHE["nc"] = _build_program()
    return _CACHE["nc"]


def kernel(x, w_qkv, w_out, b_out):
    from concourse.bass_utils import run_bass_kernel_spmd

    nc = _get_program()
    x = np.ascontiguousarray(np.asarray(x, dtype=np.float32))
    w_qkv = np.ascontiguousarray(np.asarray(w_qkv, dtype=np.float32))
    w_out = np.ascontiguousarray(np.asarray(w_out, dtype=np.float32))
    b_row = np.ascontiguousarray(np.asarray(b_out, dtype=np.float32).reshape(1, C))
    in_maps = [
        {"x": x[i], "w_qkv": w_qkv, "w_out": w_out, "b_out": b_row} for i in range(B)
    ]
    res = run_bass_kernel_spmd(nc, in_maps, core_ids=list(range(B))).results
    return np.stack([res[i]["out"] for i in range(B)], axis=0)


# revision 20
# speedup vs baseline: 1.1493x; 1.1493x over previous
"""Multi-head attention block for Trainium2, 8-core data-parallel SPMD.

Computes, per batch element b (one NeuronCore each):
    qkv = x @ w_qkv ; q,k,v split into 16 heads of dim 64
    attn = softmax(q @ k^T / sqrt(64)) ; out = (attn @ v) @ w_out + b_out

Strategy (per core):
  - transpose x -> xT (c-major) via PE transposes
  - v computed in natural layout, written strided into v_aug tiles with a
    ones-column per head so the attention output matmul also produces the
    softmax row-sums for free
  - attention per head in transposed layout: s^T = kT^T @ qT on the PE,
    exp on ACT (1/8 scale folded in), o^T_aug accumulated over k chunks;
    softmax normalization deferred to o^T (DVE reciprocal + K=1 ones-matmul
    partition-broadcast)
  - the qT/kT projection of head pair t+1 is explicitly interleaved into
    the attention instruction stream of pair t (engines execute their
    streams in order, so overlap has to be emitted, not just scheduled)
  - out = o^T^T @ w_out + ones x b_out (bias added by the PE)
All matmul-feeding tiles are declared float32r (full PE rate; the producing
DVE/ACT/DMA instructions emit the FP32r rounding the BIR verifier requires).
"""

import sys

if "/opt/trn_rl_repo" not in sys.path:
    sys.path.insert(0, "/opt/trn_rl_repo")

import numpy as np

B = 8
N = 1024  # sequence length
C = 1024  # model dim
H = 16  # heads
D = 64  # head dim
P = 128  # partitions
NT = N // P  # seq chunks
CT = C // P  # channel chunks
HP = H // 2  # head pairs
SCALE = D ** -0.5
HF = C // 512  # free-dim halves per 1024 row

_CACHE = {}


def _build_program():
    from concourse import bacc, mybir
    import concourse.tile as tile
    from concourse.masks import make_identity

    f32 = mybir.dt.float32
    f32r = mybir.dt.float32r
    Exp = mybir.ActivationFunctionType.Exp

    nc = bacc.Bacc("TRN2", target_bir_lowering=False, debug=False)
    x_d = nc.declare_dram_parameter("x", [N, C], f32r, isOutput=False)
    wqkv_d = nc.declare_dram_parameter("w_qkv", [C, 3 * C], f32r, isOutput=False)
    wout_d = nc.declare_dram_parameter("w_out", [C, C], f32r, isOutput=False)
    bout_d = nc.declare_dram_parameter("b_out", [1, C], f32r, isOutput=False)
    out_d = nc.declare_dram_parameter("out", [N, C], f32, isOutput=True)

    with tile.TileContext(nc) as tc:
        with (
            tc.tile_pool(name="consts", bufs=1) as consts,
            tc.tile_pool(name="xTo", bufs=CT) as xT_pool,
            tc.tile_pool(name="vaug", bufs=NT) as vaug_pool,
            tc.tile_pool(name="psum", bufs=1, space="PSUM") as psum,
            tc.tile_pool(name="oTp", bufs=CT) as oT_pool,
            tc.tile_pool(name="io", bufs=3) as io_pool,
            tc.tile_pool(name="w", bufs=CT) as w_pool,
            tc.tile_pool(name="wqk", bufs=4) as wqk_pool,
            tc.tile_pool(name="pT", bufs=8) as pT_pool,
            tc.tile_pool(name="recip", bufs=1) as recip_pool,
            tc.tile_pool(name="bcs", bufs=1) as bcs_pool,
            tc.tile_pool(name="qkT", bufs=4) as qkT_pool,
        ):
            identity_f32 = consts.tile(
                [P, P], f32, name="identity_f32", tag="identity_f32"
            )
            make_identity(nc, identity_f32)
            # f32r transpose runs 1.5 PE cycles/row vs 2.0 for f32
            identity = consts.tile([P, P], f32r, name="identity", tag="identity")
            nc.vector.tensor_copy(identity[:, :], identity_f32[:, :])
            # memset can't emit f32r (ISA check) — stage in f32, round via copy
            ones_f32 = consts.tile([P, P], f32, name="ones_f32", tag="ones_f32")
            nc.vector.memset(ones_f32, 1.0)
            ones = consts.tile([1, P], f32r, name="ones", tag="ones")
            nc.vector.tensor_copy(ones[0:1, :], ones_f32[0:1, :])
            b_row = consts.tile([1, C], f32r, name="b_row", tag="b_row")
            nc.sync.dma_start(out=b_row[0:1, :], in_=bout_d[0:1, :])

            xT = [
                xT_pool.tile([P, N], f32r, name=f"xT{i}", tag="xTo") for i in range(CT)
            ]
            vaug = [
                vaug_pool.tile([P, H * (D + 1)], f32r, name=f"vaug{i}", tag="vaug")
                for i in range(NT)
            ]

            def mm_tile(name, tag, bufs):
                return psum.tile([P, C], f32, name=name, tag=tag, bufs=bufs)

            def half_tile(name, tag, bufs):
                return psum.tile([P, 512], f32, name=name, tag=tag, bufs=bufs)

            # ---------------- phase 0: transpose x into xT ----------------
            for si in range(NT):
                xin = io_pool.tile([P, C], f32r, name=f"xin{si}", tag="io")
                nc.sync.dma_start(out=xin[:, :], in_=x_d[si * P : (si + 1) * P, :])
                tr_ps = psum.tile([P, C], f32r, name=f"tr{si}", tag="mm", bufs=2, padded_shape=None)
                for ci in range(CT):
                    nc.tensor.transpose(
                        tr_ps[:, ci * P : (ci + 1) * P],
                        xin[:, ci * P : (ci + 1) * P],
                        identity,
                    )
                for ci in range(CT):
                    nc.vector.tensor_copy(
                        xT[ci][:, si * P : (si + 1) * P],
                        tr_ps[:, ci * P : (ci + 1) * P],
                    )

            # ---------- phase 1B: v (natural layout) -> v_aug ----------
            # 4 seq-chunks per pass (2x [P,C] from mm/acc tags + 2 halves in
            # the sT slots) -> w_v rows streamed only twice.
            for sc0 in range(0, NT, 4):
                scs = list(range(sc0, sc0 + 4))
                full = {scs[0]: mm_tile(f"vps{scs[0]}", "mm", 2),
                        scs[1]: mm_tile(f"vps{scs[1]}", "mm", 2),
                        scs[2]: mm_tile(f"vps{scs[2]}", "acc", 1)}
                sc3 = scs[3]
                halves = [
                    half_tile(f"vps{sc3}_0", "sT", 2),
                    half_tile(f"vps{sc3}_1", "sT", 2),
                ]
                for ci in range(CT):
                    wv = w_pool.tile([P, C], f32r, name=f"wv{sc0}_{ci}", tag="w")
                    nc.sync.dma_start(
                        out=wv[:, :],
                        in_=wqkv_d[ci * P : (ci + 1) * P, 2 * C : 3 * C],
                    )
                    st = dict(start=(ci == 0), stop=(ci == CT - 1))
                    for hf in range(HF):
                        sl = slice(hf * 512, hf * 512 + 512)
                        for sc in scs[:3]:
                            nc.tensor.matmul(
                                full[sc][:, sl],
                                xT[ci][:, sc * P : (sc + 1) * P],
                                wv[:, sl],
                                **st,
                            )
                        nc.tensor.matmul(
                            halves[hf][:, :],
                            xT[ci][:, sc3 * P : (sc3 + 1) * P],
                            wv[:, sl],
                            **st,
                        )
                for sc in scs[:3]:
                    va3 = vaug[sc].rearrange("p (h u) -> p h u", u=D + 1)
                    nc.vector.tensor_copy(
                        va3[:, :, D : D + 1],
                        ones_f32[:, 0:H].rearrange("p (h u) -> p h u", u=1),
                    )
                    nc.vector.tensor_copy(
                        va3[:, :, 0:D],
                        full[sc].rearrange("p (h u) -> p h u", u=D),
                    )
                va3 = vaug[sc3].rearrange("p (h u) -> p h u", u=D + 1)
                nc.vector.tensor_copy(
                    va3[:, :, D : D + 1],
                    ones_f32[:, 0:H].rearrange("p (h u) -> p h u", u=1),
                )
                for hf in range(HF):
                    nc.vector.tensor_copy(
                        va3[:, 8 * hf : 8 * hf + 8, 0:D],
                        halves[hf].rearrange("p (h u) -> p h u", u=D),
                    )

            # ---- interleaved: attention pair t || qT/kT projection pair t+1 ----
            def qkv_pair_steps(t, qTt, kTt, q_ps, k_ps):
                """Generator: one ci-step (2 weight DMAs + 4 matmuls) per next();
                finishes with the PSUM->SBUF copies."""
                for ci in range(CT):
                    wq = wqk_pool.tile([P, P], f32r, name=f"wq{t}_{ci}", tag="wqk")
                    nc.sync.dma_start(
                        out=wq[:, :],
                        in_=wqkv_d[ci * P : (ci + 1) * P, t * P : (t + 1) * P],
                    )
                    wk = wqk_pool.tile([P, P], f32r, name=f"wk{t}_{ci}", tag="wqk")
                    nc.sync.dma_start(
                        out=wk[:, :],
                        in_=wqkv_d[ci * P : (ci + 1) * P, C + t * P : C + (t + 1) * P],
                    )
                    st = dict(start=(ci == 0), stop=(ci == CT - 1))
                    for hf in range(HF):
                        sl = slice(hf * 512, hf * 512 + 512)
                        nc.tensor.matmul(q_ps[:, sl], wq[:, :], xT[ci][:, sl], **st)
                        nc.tensor.matmul(k_ps[:, sl], wk[:, :], xT[ci][:, sl], **st)
                    yield
                nc.vector.tensor_copy(qTt[:, :], q_ps[:, :])
                nc.vector.tensor_copy(kTt[:, :], k_ps[:, :])
                yield

            def new_pair_qkv(t):
                qTt = qkT_pool.tile([P, N], f32r, name=f"qT{t}", tag="qkT")
                kTt = qkT_pool.tile([P, N], f32r, name=f"kT{t}", tag="qkT")
                q_ps = mm_tile(f"qps{t}", "mm", 2)
                k_ps = mm_tile(f"kps{t}", "mm", 2)
                return qTt, kTt, qkv_pair_steps(t, qTt, kTt, q_ps, k_ps)

            oT = [
                oT_pool.tile([P, N], f32r, name=f"oT{i}", tag="oTp")
                for i in range(CT)
            ]

            # prologue: pair 0 projection emitted straight
            qT_cur, kT_cur, gen0 = new_pair_qkv(0)
            for _ in gen0:
                pass

            # w_out is prefetched one row-chunk per head pair (inside the
            # pair loop) so the DMAs spread across the attention region
            wos = []

            def prefetch_wo(ci):
                wo = w_pool.tile([P, C], f32r, name=f"wo{ci}", tag="w")
                nc.sync.dma_start(out=wo[:, :], in_=wout_d[ci * P : (ci + 1) * P, :])
                wos.append(wo)

            pending_norm = None
            for t in range(HP):
                prefetch_wo(t)
                if t + 1 < HP:
                    qT_nxt, kT_nxt, gen = new_pair_qkv(t + 1)
                else:
                    qT_nxt = kT_nxt = gen = None
                chunk_idx = 0
                NCH = NT * HF  # 16 chunks per head
                LAG = 4  # o^T matmuls trail s/exp by LAG chunks so the
                # previous head's normalize chain hides inside the stream
                for j in range(2):
                    h = 2 * t + j
                    row0 = D * j
                    acc = mm_tile(f"acc{h}", "acc", 1)

                    def ot_mm(c, acc=acc, h=h):
                        kc, hf = divmod(c, HF)
                        sl = slice(hf * 512, hf * 512 + 512)
                        nc.tensor.matmul(
                            acc[0 : D + 1, sl],
                            vaug[kc][:, h * (D + 1) : (h + 1) * (D + 1)],
                            pts[c][:, :],
                            start=(kc == 0),
                            stop=(kc == NT - 1),
                        )

                    pts = {}
                    for c in range(NCH):
                        kc, hf = divmod(c, HF)
                        sl = slice(hf * 512, hf * 512 + 512)
                        s_ps = half_tile(f"s{h}_{kc}_{hf}", "sT", 2)
                        nc.tensor.matmul(
                            s_ps[:, :],
                            kT_cur[row0 : row0 + D, kc * P : (kc + 1) * P],
                            qT_cur[row0 : row0 + D, sl],
                            start=True,
                            stop=True,
                        )
                        pt = pT_pool.tile(
                            [P, 512], f32r, name=f"pt{h}_{kc}_{hf}", tag="pT"
                        )
                        nc.scalar.activation(
                            out=pt[:, :], in_=s_ps[:, :], func=Exp, scale=SCALE
                        )
                        pts[c] = pt
                        if c == LAG - 2 and pending_norm is not None:
                            pending_norm()
                            pending_norm = None
                        if c >= LAG:
                            ot_mm(c - LAG)
                            del pts[c - LAG]
                        # sprinkle next pair's projection into the stream
                        if gen is not None and chunk_idx % 3 == 2:
                            next(gen, None)
                        chunk_idx += 1
                    for c in range(NCH - LAG, NCH):
                        ot_mm(c)

                    def normalize(h=h, row0=row0, t=t, acc=acc):
                        # o^T[d, q] *= 1 / rowsum[q]
                        rc = recip_pool.tile([1, N], f32r, name=f"rc{h}", tag="recip")
                        with nc.allow_low_precision(
                            reason="softmax norm reciprocal rounded to f32r "
                            "for the PE broadcast matmul"
                        ):
                            nc.vector.reciprocal(rc[0:1, :], acc[D : D + 1, :])
                        bcs = bcs_pool.tile([D, N], f32, name=f"bcs{h}", tag="bcs")
                        for hf in range(HF):
                            sl = slice(hf * 512, hf * 512 + 512)
                            bc = half_tile(f"bc{h}_{hf}", "sT", 2)
                            nc.tensor.matmul(
                                bc[0:D, :],
                                ones[0:1, 0:D],
                                rc[0:1, sl],
                                start=True,
                                stop=True,
                            )
                            # DVE reads at most one PSUM operand: stage in SBUF
                            nc.vector.tensor_copy(bcs[0:D, sl], bc[0:D, :])
                        nc.vector.tensor_mul(
                            oT[t][row0 : row0 + D, :],
                            acc[0:D, :],
                            bcs[0:D, :],
                        )

                    pending_norm = normalize
                if gen is not None:
                    for _ in gen:
                        pass
                qT_cur, kT_cur = qT_nxt, kT_nxt
            pending_norm()  # last head's normalize

            # ---------------- phase 3: out = o @ w_out + b ----------------
            for sc in range(NT):
                o_ps = mm_tile(f"ops{sc}", "mm", 2)
                for ci in range(CT):
                    for hf in range(HF):
                        sl = slice(hf * 512, hf * 512 + 512)
                        nc.tensor.matmul(
                            o_ps[:, sl],
                            oT[ci][:, sc * P : (sc + 1) * P],
                            wos[ci][:, sl],
                            start=(ci == 0),
                            stop=False,
                        )
                for hf in range(HF):
                    sl = slice(hf * 512, hf * 512 + 512)
                    nc.tensor.matmul(
                        o_ps[:, sl],
                        ones[0:1, 0:P],
                        b_row[0:1, sl],
                        start=False,
                        stop=True,
                    )
                ot = io_pool.tile([P, C], f32, name=f"ot{sc}", tag="io")
                nc.vector.tensor_copy(ot[:, :], o_ps[:, :])
                nc.sync.dma_start(out=out_d[sc * P : (sc + 1) * P, :], in_=ot[:, :])

    nc.compile()
    return nc


def _get_program():
    if "nc" not in _CACHE:
        _CACHE["nc"] = _build_program()
    return _CACHE["nc"]


def kernel(x, w_qkv, w_out, b_out):
    from concourse.bass_utils import run_bass_kernel_spmd

    nc = _get_program()
    x = np.ascontiguousarray(np.asarray(x, dtype=np.float32))
    w_qkv = np.ascontiguousarray(np.asarray(w_qkv, dtype=np.float32))
    w_out = np.ascontiguousarray(np.asarray(w_out, dtype=np.float32))
    b_row = np.ascontiguousarray(np.asarray(b_out, dtype=np.float32).reshape(1, C))
    in_maps = [
        {"x": x[i], "w_qkv": w_qkv, "w_out": w_out, "b_out": b_row} for i in range(B)
    ]
    res = run_bass_kernel_spmd(nc, in_maps, core_ids=list(range(B))).results
    return np.stack([res[i]["out"] for i in range(B)], axis=0)
